# revision 51
# baseline (speedup 1.0000x reference)
"""Trainium2 Bass kernel for nn_Action_37890201485804 (scatter_memory).

Pointer-generator style head:
  gen logits = dec_out @ gen_W + gen_b            [B, LA, V]
  copy logits = dec_out @ src_hidden^T (masked)   [B, LA, S]
  probs = softmax(concat * H^-0.5)
  out = gen_p + probs_copy @ copy_sources (+ scatter_add of ctx block)

Strategy: pure data parallel over batch B across the 8 NeuronCores
(8 batches per core).  Everything runs on-device except index/layout
prep: the host pre-transposes dec_out / src_hidden (layout only), sorts
each batch's scatter indices into fixed slot ranges (`slot` rows per
512-wide v-tile) and passes small int16 index tensors.  The scatter_add
becomes a matmul against a one-hot matrix built on-device by comparing
an iota against per-partition keys; softmax probabilities are routed
into slot order with one ap_gather (whose per-16-partition index groups
give per-batch gather maps -- each batch owns a 16-row group).

Row layout on each core: partition 16*b + l for local batch b, decoder
step l (l >= 8 rows are masked padding).  The gen projection runs in
fp8 e4m3 (gen_W*128, dec*16, scales refolded into the Exp) with
DoubleRow K=256 matmuls -- halving both the dominant HBM stream
(16.8MB -> 8.4MB per core) and the PE passes.  The scatter matmul is
also fp8 DoubleRow: both 128-row slot chunks of a v-tile fold into one
K=256 matmul against fp8 one-hots; gen probs join the same PSUM via an
identity matmul and ScalarE evacuates with 1/sum as its per-partition
activation scale.  Output is written bf16 and upcast on host.

Softmax runs without max subtraction (post-scale logits here are O(5)):
each gen PSUM tile goes straight through one ScalarE Exp (scale and a
-ln16 bias folded in, accum_out collecting the denominator) into a bf16
unnormalized-probs/16 buffer -- the /16 keeps every unnormalized prob
inside fp8 e4m3 range and cancels exactly through the shared 1/sum.

Schedule: the copy block + its 2x2MB coalesced src DMAs go first on the
sync ring so probs_cb, the ap_gather, and the fixed ~30us gpsimd
pre-gather drain all hide under the fp8 gen stream.

When pv_m / l / tp / related are exact one-hot matrices (they are for
this module -- "one_hot_scatter"), their einsums are folded into the
same slot-scatter machinery (fast=True) and those matrices are never
read on device.  Otherwise a dense path DMAs them as matmul operands.
"""

import sys

sys.path.insert(0, "/opt/trn_rl_repo")

import numpy as np
import ml_dtypes

BF = ml_dtypes.bfloat16

import concourse.bass as bass  # noqa: F401  (engine classes)
import concourse.tile as tile
from concourse import bacc, mybir
from concourse.bass_utils import run_bass_kernel_spmd
from concourse import library_config

# ---------------------------------------------------------------- constants
B, LA, H, V = 64, 8, 512, 16384
PREF, PROF, STATE, CTX, REL = 10, 10, 10, 256, 30
S = PREF + PROF + STATE + CTX + REL  # 316
NEG = -1e9
SCALE = float(H) ** -0.5

NCORE = 8
BL = B // NCORE          # local batches per core (8)
TS = 512                 # v-tile width
NT = V // TS             # 32 v-tiles
TG = 4                   # v-tiles per gen_W DMA group
PADCOL = 60              # a guaranteed masked pad column (prob == 0)
F32 = mybir.dt.float32
BF16 = mybir.dt.bfloat16
FP8 = mybir.dt.float8e4
I16 = mybir.dt.int16
U16 = mybir.dt.uint16
NEG_S = NEG * SCALE
LN16 = float(np.log(16.0))   # exp bias: keeps unnormalized probs fp8-safe
F8 = ml_dtypes.float8_e4m3
F16 = mybir.dt.float16
SW = 128.0                # fp8 scale on gen_W
SD = 16.0                 # fp8 scale on dec (gen path)
EXP_GEN = SCALE / (SW * SD)

_BUILD_CACHE: dict = {}


# ================================================================ builder
def _build(slot: int, genb_nz: bool, fast: bool, csw: int, dgat: int):
    """Build + compile the 8-core SPMD graph.

    slot: scatter slot rows per (batch, v-tile) -- 32 or 64.
    fast: pv/l/tp/related are one-hot and folded into the scatter path.
    csw:  compact copy-space width (<= 512).
    dgat: gather block size (8 or 4).
    """
    key = (slot, genb_nz, fast, csw, dgat)
    if key in _BUILD_CACHE:
        return _BUILD_CACHE[key]
    CSW = csw

    psw = BL * slot * NT         # slot-space width (8192 for slot=32)
    cpt = BL * slot // 128       # 128-row transpose chunks per v-tile (2 or 4)
    cpq = cpt // 2               # chunks per (v-tile, quad)

    nc = bacc.Bacc(
        "TRN2", target_bir_lowering=False, debug=False, num_devices=NCORE,
    )

    def din(name, shape, dtype=F32):
        return nc.dram_tensor(name, list(shape), dtype, kind="ExternalInput").ap()

    gen_w8 = din("gen_w8", (128, 4, V), FP8)      # gen_W * SW, [p, kc, v]
    dec_g8 = din("dec_g8", (128, 4, 128), FP8)    # dec^T * SD, [p, kc, (b,l16)]
    if genb_nz:
        genb = din("genb", (1, V), BF16)
    dec_cl = din("dec_cl", (128, 2048), BF16)     # quad-block-diag dec chunks
    src_cs8 = din("src_cs8", (128, 32, CSW), BF16)  # src^T blocks [p, (q,i,kc), c]
    maskadd = din("maskadd", (128, CSW), BF16)    # 0 / NEG*SCALE additive mask
    gblk = din("gblk", (128, psw // dgat // 16), I16)  # gather block ids
    tw = din("tw", (128, NT * cpt))          # one-hot keys per slot chunk
    iota_in = din("iota_in", (128, TS), F16)      # 0..511 per partition
    ident_in = din("ident_in", (128, 128))        # fp32 identity (transposes)
    if not fast:
        c30 = din("c30", (BL * 30, V), BF16)      # [pv;l;tp] concat per batch
        crel = din("crel", (BL * REL, V), BF16)
    out = nc.dram_tensor("out", [128, V], BF16, kind="ExternalOutput").ap()

    with tile.TileContext(nc) as tc:
        with (
            tc.tile_pool(name="const", bufs=1) as constp,
            tc.tile_pool(name="pgen", bufs=1) as pgenp,
            tc.tile_pool(name="decs", bufs=1) as decp,
            tc.tile_pool(name="genw", bufs=6) as genwp,
            tc.tile_pool(name="cat", bufs=4) as catp,
            tc.tile_pool(name="ohs", bufs=16) as ohsp,
            tc.tile_pool(name="probs", bufs=1) as probsp,
            tc.tile_pool(name="ptcs", bufs=12) as ptcp,
            tc.tile_pool(name="outs", bufs=4) as outp,
            tc.tile_pool(name="gen_ps", bufs=2, space="PSUM") as genps,
            tc.tile_pool(name="copy_ps", bufs=3, space="PSUM") as copyps,
            tc.tile_pool(name="cl_ps", bufs=1, space="PSUM") as clps,
            tc.tile_pool(name="tr_ps", bufs=2, space="PSUM") as trps,
        ):
            # preload the Q7 ucode library for ap_gather so the ~130us
            # reload overlaps the gen phase instead of gating the main loop
            nc.gpsimd.load_library(library_config.ap_gather)

            # dummy ap_gather: absorbs the ~30us gpsimd pre-op drain at t=0
            # (GpSimd queue only) so the real gather isn't gated by it
            dgi = constp.tile([16, 1], I16, tag="dgi")
            nc.gpsimd.memset(dgi[:], 0)
            dgd = constp.tile([16, 16], BF16, tag="dgd")
            nc.gpsimd.memset(dgd[:], 0.0)
            dgo = constp.tile([16, 32], BF16, tag="dgo")
            nc.gpsimd.ap_gather(
                dgo[:], dgd[:], dgi[:],
                channels=16, num_elems=8, d=2, num_idxs=16,
            )

            # ---------------- DMA order: copy-block feeders first (they
            # gate probs_cb -> gather -> the whole scatter pipeline)
            deccl_sb = decp.tile([128, 2048], BF16, tag="deccl")
            nc.sync.dma_start(deccl_sb[:], dec_cl[:, :])

            sums = constp.tile([128, NT + 1], F32, tag="sums")
            pgen = pgenp.tile([128, V], BF16)      # unnormalized gen probs / 16

            ln16n = constp.tile([128, 1], F32, tag="ln16n")
            nc.vector.memset(ln16n[:], -LN16)
            ones_sb = decp.tile([1, 128], BF16, tag="ones")
            nc.vector.memset(ones_sb[:], 1.0)

            # ---------------- remaining small inputs (all ahead of wt/src)
            dec8_sb = decp.tile([128, 4, 128], FP8, tag="dec8")
            nc.sync.dma_start(dec8_sb[:], dec_g8[:, :, :])
            maskadd_sb = decp.tile([128, CSW], BF16, tag="maskadd")
            nc.sync.dma_start(maskadd_sb[:], maskadd[:, :])
            gblk_sb = decp.tile([128, psw // dgat // 16], I16, tag="gblk")
            nc.sync.dma_start(gblk_sb[:], gblk[:, :])
            tw_sb = decp.tile([128, NT * cpt], F32, tag="tw")
            nc.sync.dma_start(tw_sb[:], tw[:, :])
            iota512 = constp.tile([128, TS], F16)
            nc.sync.dma_start(iota512[:], iota_in[:, :])
            ident = constp.tile([128, 128], F32)
            nc.sync.dma_start(ident[:], ident_in[:, :])
            identb = constp.tile([128, 128], BF16, tag="identb")
            nc.vector.tensor_copy(identb[:], ident[:])

            # gen probs: per tile-group DMA, DoubleRow fp8 matmuls,
            # then Exp straight off PSUM with the prefix scale folded in.
            for g in range(NT // TG):
                wt = genwp.tile([128, 4, TG * TS], FP8, tag="w")
                nc.sync.dma_start(
                    wt[:], gen_w8[:, :, TG * TS * g:TG * TS * (g + 1)])
                if genb_nz:
                    bt = genwp.tile([1, TG * TS], BF16, tag="bias")
                    nc.sync.dma_start(
                        bt[:], genb[0:1, TG * TS * g:TG * TS * (g + 1)])
                for tt in range(TG):
                    t = TG * g + tt
                    ps = genps.tile([128, TS], F32, tag="gen")
                    for k in range(2):
                        nc.tensor.matmul(
                            ps[:], dec8_sb[:, 2 * k:2 * k + 2, :],
                            wt[:, 2 * k:2 * k + 2, TS * tt:TS * (tt + 1)],
                            start=(k == 0), stop=(k == 1 and not genb_nz),
                            perf_mode=mybir.MatmulPerfMode.DoubleRow,
                        )
                    if genb_nz:
                        # genb pre-scaled by SW*SD on host to match the
                        # fp8 product scale in PSUM
                        nc.tensor.matmul(
                            ps[:], ones_sb[:], bt[:, TS * tt:TS * (tt + 1)],
                            start=False, stop=True,
                        )
                    nc.scalar.activation(
                        pgen[:, TS * t:TS * (t + 1)], ps[:],
                        mybir.ActivationFunctionType.Exp,
                        scale=EXP_GEN,
                        bias=ln16n[:, 0:1],
                        accum_out=sums[:, t:t + 1],
                    )

            # ---------------- copy block: src streams AFTER gen_w8 so the
            # ~30us gpsimd drain that precedes ap_gather (which freezes
            # the whole DMA subsystem) fires only once all input DMA has
            # drained -- the gen phase never starves.
            with tc.tile_pool(name="srcs", bufs=2) as srcp:
                cps = clps.tile([128, CSW], F32)
                src_w = []
                for q in range(2):
                    sw_ = srcp.tile([128, 16, CSW], BF16, tag="src")
                    nc.sync.dma_start(
                        sw_[:], src_cs8[:, 16 * q:16 * (q + 1), :])
                    src_w.append(sw_)
                for q in range(2):
                    for kc in range(16):
                        nc.tensor.matmul(
                            cps[64 * q:64 * (q + 1), :],
                            deccl_sb[:, q * 1024 + kc * 64:
                                     q * 1024 + (kc + 1) * 64],
                            src_w[q][:, kc, :],
                            start=(kc == 0), stop=(kc == 15),
                        )
                # copy-block logits then unnormalized probs/16 + denominator
                lcb = probsp.tile([128, CSW], F32, tag="lcb")
                nc.vector.scalar_tensor_tensor(
                    lcb[:], cps[:], SCALE, maskadd_sb[:],
                    op0=mybir.AluOpType.mult, op1=mybir.AluOpType.add,
                )
                probs_cb = probsp.tile([128, CSW], BF16, tag="pcb")
                nc.scalar.activation(
                    probs_cb[:], lcb[:],
                    mybir.ActivationFunctionType.Exp,
                    bias=ln16n[:, 0:1],
                    accum_out=sums[:, NT:NT + 1],
                )


            # slot-gather the (unnormalized) copy-block probs; native pool op
            gath = probsp.tile([128, psw], BF16, tag="gath")
            nc.gpsimd.ap_gather(
                gath[:], probs_cb[:], gblk_sb[:],
                channels=128, num_elems=CSW // dgat, d=dgat,
                num_idxs=psw // dgat,
            )

            # ---------------- denominator (of the /16-scaled exps)
            sumexp = constp.tile([128, 1], F32, tag="sumexp")
            nc.vector.tensor_reduce(
                sumexp[:], sums[:], axis=mybir.AxisListType.X,
                op=mybir.AluOpType.add,
            )
            recip = constp.tile([128, 1], F32, tag="recip")
            nc.vector.reciprocal(recip[:], sumexp[:])

            if not fast:
                # 1/sum folded into a diagonal for the legacy transpose path
                diagr = constp.tile([128, 128], BF16, tag="diagr")
                nc.vector.tensor_scalar_mul(diagr[:], ident[:], recip[:])
                # quad-block-diag lhsTs for the fixed-60 dense matmuls
                tp0 = trps.tile([128, 128], F32, tag="tr")
                nc.tensor.matmul(tp0[0:64, :], probs_cb[:, 0:64], diagr[:])
                ptfix = probsp.tile([64, 128], BF16, tag="ptfix")
                nc.vector.tensor_copy(ptfix[:], tp0[0:64, :])
                bd30 = probsp.tile([128, 128], BF16, tag="bd30")
                bdrel = probsp.tile([128, 128], BF16, tag="bdrel")
                nc.vector.memset(bd30[:], 0.0)
                nc.vector.memset(bdrel[:], 0.0)
                for q in range(2):
                    for i in range(4):
                        cs = 64 * q + 16 * i
                        nc.sync.dma_start(
                            bd30[30 * i:30 * i + 30, cs:cs + 16],
                            ptfix[0:30, cs:cs + 16])
                        nc.sync.dma_start(
                            bdrel[30 * i:30 * i + 30, cs:cs + 16],
                            ptfix[30:60, cs:cs + 16])

            # ---------------- main output loop
            if fast:
                # fp8 DoubleRow scatter: both 128-row slot chunks of a tile
                # fold into ONE K=256 matmul; gen probs accumulate into the
                # same PSUM via an identity matmul; ScalarE evacuates with
                # the 1/sum folded into its per-partition scale.
                for t in range(NT):
                    cp = copyps.tile([128, TS], F32, tag="cp")
                    ptc2 = ptcp.tile([128, cpt, 128], FP8, tag="ptc")
                    o2 = ohsp.tile([128, cpt, TS], FP8, tag="ohs")
                    for cc in range(cpt):
                        tpp = trps.tile([128, 128], F32, tag="tr")
                        koff = 128 * (t * cpt + cc)
                        nc.tensor.matmul(
                            tpp[:], gath[:, koff:koff + 128], identb[:])
                        if cc % 2 == 0:
                            nc.scalar.copy(ptc2[:, cc, :], tpp[:])
                        else:
                            nc.vector.tensor_copy(ptc2[:, cc, :], tpp[:])
                        nc.vector.tensor_scalar(
                            o2[:, cc, :], iota512[:],
                            tw_sb[:, t * cpt + cc:t * cpt + cc + 1],
                            None, mybir.AluOpType.is_equal,
                        )
                    for pp in range(cpt // 2):
                        nc.tensor.matmul(
                            cp[:], ptc2[:, 2 * pp:2 * pp + 2, :],
                            o2[:, 2 * pp:2 * pp + 2, :],
                            start=(pp == 0), stop=False,
                            perf_mode=mybir.MatmulPerfMode.DoubleRow,
                        )
                    nc.tensor.matmul(
                        cp[:], identb[:], pgen[:, TS * t:TS * (t + 1)],
                        start=False, stop=True,
                    )
                    ot = outp.tile([128, TS], BF16, tag="o")
                    nc.scalar.mul(ot[:], cp[:], recip[:, 0:1])
                    nc.sync.dma_start(out[:, TS * t:TS * (t + 1)], ot[:])

            for t in range(0 if fast else NT):    # legacy (dense) path
                cp = copyps.tile([128, TS], F32, tag="cp")
                for q in range(2):
                    if not fast:
                        cat30 = catp.tile([128, TS], BF16, tag="cat30")
                        catrel = catp.tile([128, TS], BF16, tag="catrel")
                        nc.sync.dma_start(
                            cat30[0:120, :],
                            c30[120 * q:120 * (q + 1), TS * t:TS * (t + 1)])
                        nc.sync.dma_start(
                            catrel[0:120, :],
                            crel[120 * q:120 * (q + 1), TS * t:TS * (t + 1)])
                        nc.tensor.matmul(
                            cp[64 * q:64 * (q + 1), :],
                            bd30[0:120, 64 * q:64 * (q + 1)],
                            cat30[0:120, :],
                            start=True, stop=False,
                        )
                        nc.tensor.matmul(
                            cp[64 * q:64 * (q + 1), :],
                            bdrel[0:120, 64 * q:64 * (q + 1)],
                            catrel[0:120, :],
                            start=False, stop=False,
                        )
                    for s in range(cpq):
                        cc = q * cpq + s            # chunk within this tile
                        tpp = trps.tile([128, 128], F32, tag="tr")
                        koff = 128 * (t * cpt + cc)
                        nc.tensor.matmul(
                            tpp[:],
                            gath[:, koff:koff + 128],
                            diagr[:])
                        ptc = ptcp.tile([128, 128], BF16, tag="ptc")
                        nc.scalar.copy(ptc[:], tpp[:])
                        o = ohsp.tile([128, TS], BF16, tag="ohs")
                        nc.vector.tensor_scalar(
                            o[:], iota512[:],
                            tw_sb[:, t * cpt + cc:t * cpt + cc + 1],
                            None, mybir.AluOpType.is_equal,
                        )
                        nc.tensor.matmul(
                            cp[64 * q:64 * (q + 1), :],
                            ptc[:, 64 * q:64 * (q + 1)],
                            o[:],
                            start=(fast and s == 0), stop=(s == cpq - 1),
                        )
                # out = pgen * recip + copy_psum
                ot = outp.tile([128, TS], BF16, tag="o")
                nc.vector.scalar_tensor_tensor(
                    ot[:], pgen[:, TS * t:TS * (t + 1)], recip[:], cp[:],
                    op0=mybir.AluOpType.mult, op1=mybir.AluOpType.add,
                )
                nc.sync.dma_start(
                    out[:, TS * t:TS * (t + 1)], ot[:])

    nc.compile()
    _BUILD_CACHE[key] = nc
    return nc


# ================================================================ host prep
def _onehot_idx(mat):
    """Return [B, p] argmax indices if mat rows are exact one-hot, else None."""
    mat = np.asarray(mat)
    idx = mat.argmax(-1)
    if not (np.take_along_axis(mat, idx[..., None], -1) == 1.0).all():
        return None
    if (mat != 0).sum(-1).max() != 1:
        return None
    return idx.astype(np.int64)


def _prep(dec_out, src_hidden, src_mask, pv_m, l, tp, related,
          gen_W, gen_b, context, glo2loc):
    """Numpy-side layout/index prep -> (in_maps, slot, genb_nz, fast)."""
    f32 = np.float32
    dec_out = np.asarray(dec_out, f32)
    src_hidden = np.asarray(src_hidden, f32)
    src_mask = np.asarray(src_mask)
    gen_W = np.asarray(gen_W, f32)
    gen_b = np.asarray(gen_b, f32)
    context = np.asarray(context)
    glo2loc = np.asarray(glo2loc)

    transfer = glo2loc[context].astype(np.int64)          # [B, CTX]
    order = np.argsort(transfer, axis=1, kind="stable")   # [B, CTX]

    oh = [_onehot_idx(m) for m in (pv_m, l, tp, related)]
    fast = all(o is not None for o in oh)

    # scatter entry list per batch, sorted by target v.  Each entry's
    # compact-copy-space column equals its sorted position (+64 in the
    # dense path), so each (batch, v-tile) occupies one consecutive run
    # and the slot gather works on 32-wide blocks.
    ctx_targets = np.take_along_axis(transfer, order, 1)  # sorted ctx targets
    ctx_srcrow = 30 + order                               # original src rows
    if fast:
        fixed_t = np.concatenate(oh, 1)                   # [B, 60]
        fr = np.concatenate([np.arange(30), 286 + np.arange(30)])
        fixed_r = np.tile(fr, (B, 1))
        targets = np.concatenate([fixed_t, ctx_targets], 1)   # [B, 316]
        srcrow = np.concatenate([fixed_r, ctx_srcrow], 1)
        csbase = 0
    else:
        targets, srcrow = ctx_targets, ctx_srcrow
        csbase = 64
    o2 = np.argsort(targets, axis=1, kind="stable")
    targets = np.take_along_axis(targets, o2, 1)
    srcrow = np.take_along_axis(srcrow, o2, 1)
    ne = targets.shape[1]

    tile_of = targets // TS
    within = (targets % TS).astype(np.float32)
    K = np.zeros((B, NT), np.int64)
    np.add.at(K, (np.repeat(np.arange(B), ne), tile_of.ravel()), 1)

    slot = 32 if K.max() <= 32 else 64
    assert K.max() <= slot, f"scatter tile count {K.max()} > {slot}"
    psw = BL * slot * NT
    cpt = BL * slot // 128

    # pad each (batch, tile) run to a multiple of the gather block size
    # so runs start block-aligned in the compact copy space
    dgat = 8
    L = (np.ceil(K / dgat) * dgat).astype(np.int64)       # padded run lengths
    if int(L.sum(1).max()) + 32 > 512 - csbase:
        dgat = 4
        L = (np.ceil(K / dgat) * dgat).astype(np.int64)
    cum8 = np.concatenate(
        [np.zeros((B, 1), np.int64), np.cumsum(L, 1)[:, :-1]], 1) + csbase
    csw = 512
    assert int((cum8[:, -1] + L[:, -1]).max()) + 32 <= csw

    genb_nz = bool(np.any(gen_b != 0.0))

    # per-batch column map: sorted entry i -> its padded-run column
    colpos = np.zeros((B, ne), np.int64)
    for b in range(B):
        rank = np.arange(ne) - np.concatenate(
            [np.zeros(1, np.int64),
             np.cumsum(K[b])])[tile_of[b]]
        colpos[b] = cum8[b][tile_of[b]] + rank

    # src_hidden^T in compact copy-space order + additive mask
    srcT = src_hidden.transpose(0, 2, 1)                  # [B, H, S]
    src_cs = np.zeros((B, H, csw), f32)
    m = src_mask[:, 0, :]                                 # [B, S]
    maskcs = np.zeros((B, csw), np.int64)
    if not fast:
        src_cs[:, :, 0:30] = srcT[:, :, 0:30]
        src_cs[:, :, 30:60] = srcT[:, :, 286:316]
        maskcs[:, 0:30] = m[:, 0:30]
        maskcs[:, 30:60] = m[:, 286:316]
    bi = np.repeat(np.arange(B), ne)
    src_cs[bi, :, colpos.ravel()] = np.take_along_axis(
        srcT, srcrow[:, None, :], 2).transpose(0, 2, 1).reshape(B * ne, H)
    maskcs[bi, colpos.ravel()] = np.take_along_axis(m, srcrow, 1).ravel()

    # one-hot keys per (batch, tile, rank); gather block ids per tile
    tw_bts = np.full((B, NT, slot), -1, np.float32)
    for b in range(B):
        rank = colpos[b] - cum8[b][tile_of[b]]
        tw_bts[b, tile_of[b], rank] = within[b]
    gblk_b = (cum8 // dgat).astype(np.int16)              # [B, NT] block ids
    PADBLK = (csw - 32) // dgat                           # all-masked block

    iota_in = np.tile(np.arange(TS, dtype=np.float16), (128, 1))
    ident_in = np.eye(128, dtype=f32)

    in_maps = []
    # fp8 gen weights: [p, kc, v] = W[kc*128+p, v] * SW
    gen_w8 = np.ascontiguousarray(
        (gen_W.reshape(4, 128, V).transpose(1, 0, 2) * SW).astype(F8))
    genb_s = np.ascontiguousarray(
        (gen_b[None, :] * (SW * SD)).astype(BF)) if genb_nz else None
    if not fast:
        c30_all = np.concatenate(
            [np.asarray(pv_m, f32), np.asarray(l, f32),
             np.asarray(tp, f32)], 1)                     # [B, 30, V]
        crel_all = np.asarray(related, f32)
    for c in range(NCORE):
        gb = slice(c * BL, (c + 1) * BL)
        gbi = np.arange(c * BL, (c + 1) * BL)
        d = dec_out[gb]                                    # [BL, LA, H]

        dec_gx = np.zeros((H, 128), f32)
        for b in range(BL):
            dec_gx[:, 16 * b:16 * b + LA] = d[b].T
        dec_g8 = (dec_gx.reshape(4, 128, 128).transpose(1, 0, 2) * SD)

        dec_cl = np.zeros((128, 2048), f32)
        for q in range(2):
            for kc in range(16):
                lb = 4 * q + kc // 4
                hs = slice(128 * (kc % 4), 128 * (kc % 4 + 1))
                off = q * 1024 + kc * 64 + 16 * (kc // 4)
                dec_cl[:, off:off + LA] = d[lb].T[hs]

        maskadd_c = np.full((128, csw), NEG_S, f32)
        for b in range(BL):
            maskadd_c[16 * b:16 * b + LA, :] = np.where(
                maskcs[c * BL + b] == 1, 0.0, NEG_S)[None, :]

        # gather block ids: slot block k' -> tile k'//(32*8/d) ... each
        # (batch, tile) owns 32/dgat consecutive blocks; a batch's group
        # keeps its own runs, other batches' blocks point at the
        # all-masked pad block (prob == 0 -> block-diag zeros)
        bpt = slot // dgat                  # blocks per (batch, tile)
        nblk = psw // dgat
        gblk_c = np.zeros((128, nblk // 16), np.int16)
        kk2 = np.arange(nblk)
        tile2, sub2b = kk2 // (BL * bpt), kk2 % (BL * bpt)
        bat2, off2 = sub2b // bpt, sub2b % bpt
        for b in range(BL):
            lst = np.where(bat2 == b,
                           gblk_b[c * BL + b, tile2] + off2, PADBLK)
            for p in range(16):
                gblk_c[16 * b + p] = lst[p::16].astype(np.int16)

        # one-hot keys per slot chunk
        tw_c = np.full((128, NT * cpt), -1, np.float32)
        rr = np.arange(128)
        for t in range(NT):
            for cc in range(cpt):
                kk = 128 * (t * cpt + cc) + rr
                sub2 = kk % (BL * slot)
                tw_c[:, t * cpt + cc] = tw_bts[
                    gbi[sub2 // slot], t, sub2 % slot]

        im = dict(
            gen_w8=gen_w8,
            dec_g8=np.ascontiguousarray(dec_g8.astype(F8)),
            dec_cl=np.ascontiguousarray(dec_cl.astype(BF)),
            src_cs8=np.ascontiguousarray(
                src_cs[gb].reshape(2, 4, 4, 128, csw)
                .transpose(3, 0, 1, 2, 4).reshape(128, 32, csw).astype(BF)),
            maskadd=np.ascontiguousarray(maskadd_c.astype(BF)),
            gblk=np.ascontiguousarray(gblk_c),
            tw=np.ascontiguousarray(tw_c),
            iota_in=iota_in,
            ident_in=ident_in,
        )
        if genb_nz:
            im["genb"] = genb_s
        if not fast:
            im.update(
                c30=np.ascontiguousarray(
                    c30_all[gb].reshape(BL * 30, V).astype(BF)),
                crel=np.ascontiguousarray(
                    crel_all[gb].reshape(BL * REL, V).astype(BF)),
            )
        in_maps.append(im)
    return in_maps, slot, genb_nz, fast, csw, dgat


# ================================================================ entry
def kernel(**inputs) -> np.ndarray:
    in_maps, slot, genb_nz, fast, csw, dgat = _prep(**inputs)
    nc = _build(slot, genb_nz, fast, csw, dgat)
    res = run_bass_kernel_spmd(nc, in_maps, core_ids=list(range(NCORE)))
    outs = [np.asarray(res.results[c]["out"]).astype(np.float32)
            .reshape(BL, 16, V)[:, :LA]
            for c in range(NCORE)]
    return np.concatenate(outs, 0)



# revision 54
# speedup vs baseline: 1.1524x; 1.1524x over previous
"""Trainium2 Bass kernel for nn_Action_37890201485804 (scatter_memory).

Pointer-generator style head:
  gen logits = dec_out @ gen_W + gen_b            [B, LA, V]
  copy logits = dec_out @ src_hidden^T (masked)   [B, LA, S]
  probs = softmax(concat * H^-0.5)
  out = gen_p + probs_copy @ copy_sources (+ scatter_add of ctx block)

Strategy: pure data parallel over batch B across the 8 NeuronCores
(8 batches per core).  Everything runs on-device except index/layout
prep: the host pre-transposes dec_out / src_hidden (layout only), sorts
each batch's scatter indices into fixed slot ranges (`slot` rows per
512-wide v-tile) and passes small int16 index tensors.  The scatter_add
becomes a matmul against a one-hot matrix built on-device by comparing
an iota against per-partition keys; softmax probabilities are routed
into slot order with one ap_gather (whose per-16-partition index groups
give per-batch gather maps -- each batch owns a 16-row group).

Row layout on each core: partition 16*b + l for local batch b, decoder
step l (l >= 8 rows are masked padding).  The gen projection runs in
fp8 e4m3 (gen_W*128, dec*16, scales refolded into the Exp) with
DoubleRow K=256 matmuls -- halving both the dominant HBM stream
(16.8MB -> 8.4MB per core) and the PE passes.  The scatter matmul is
also fp8 DoubleRow: both 128-row slot chunks of a v-tile fold into one
K=256 matmul against fp8 one-hots; gen probs join the same PSUM via an
identity matmul and ScalarE evacuates with 1/sum as its per-partition
activation scale.  Output is written bf16 and upcast on host.

Softmax runs without max subtraction (post-scale logits here are O(5)):
each gen PSUM tile goes straight through one ScalarE Exp (scale and a
-ln16 bias folded in, accum_out collecting the denominator) into a bf16
unnormalized-probs/16 buffer -- the /16 keeps every unnormalized prob
inside fp8 e4m3 range and cancels exactly through the shared 1/sum.

Schedule: the copy block + its 2x2MB coalesced src DMAs go first on the
sync ring so probs_cb, the ap_gather, and the fixed ~30us gpsimd
pre-gather drain all hide under the fp8 gen stream.

When pv_m / l / tp / related are exact one-hot matrices (they are for
this module -- "one_hot_scatter"), their einsums are folded into the
same slot-scatter machinery (fast=True) and those matrices are never
read on device.  Otherwise a dense path DMAs them as matmul operands.
"""

import sys

sys.path.insert(0, "/opt/trn_rl_repo")

import numpy as np
import ml_dtypes

BF = ml_dtypes.bfloat16

import concourse.bass as bass  # noqa: F401  (engine classes)
import concourse.tile as tile
from concourse import bacc, mybir
from concourse.bass_utils import run_bass_kernel_spmd
from concourse import library_config

# ---------------------------------------------------------------- constants
B, LA, H, V = 64, 8, 512, 16384
PREF, PROF, STATE, CTX, REL = 10, 10, 10, 256, 30
S = PREF + PROF + STATE + CTX + REL  # 316
NEG = -1e9
SCALE = float(H) ** -0.5

NCORE = 8
BL = B // NCORE          # local batches per core (8)
TS = 512                 # v-tile width
NT = V // TS             # 32 v-tiles
TG = 4                   # v-tiles per gen_W DMA group
PADCOL = 60              # a guaranteed masked pad column (prob == 0)
F32 = mybir.dt.float32
BF16 = mybir.dt.bfloat16
FP8 = mybir.dt.float8e4
I16 = mybir.dt.int16
U16 = mybir.dt.uint16
NEG_S = NEG * SCALE
LN16 = float(np.log(16.0))   # exp bias: keeps unnormalized probs fp8-safe
F8 = ml_dtypes.float8_e4m3
F16 = mybir.dt.float16
SW = 128.0                # fp8 scale on gen_W
SD = 16.0                 # fp8 scale on dec (gen path)
EXP_GEN = SCALE / (SW * SD)

_BUILD_CACHE: dict = {}


# ================================================================ builder
def _build(slot: int, genb_nz: bool, fast: bool, csw: int, dgat: int):
    """Build + compile the 8-core SPMD graph.

    slot: scatter slot rows per (batch, v-tile) -- 32 or 64.
    fast: pv/l/tp/related are one-hot and folded into the scatter path.
    csw:  compact copy-space width (<= 512).
    dgat: gather block size (8 or 4).
    """
    key = (slot, genb_nz, fast, csw, dgat)
    if key in _BUILD_CACHE:
        return _BUILD_CACHE[key]
    CSW = csw

    psw = BL * slot * NT         # slot-space width (8192 for slot=32)
    cpt = BL * slot // 128       # 128-row transpose chunks per v-tile (2 or 4)
    cpq = cpt // 2               # chunks per (v-tile, quad)

    nc = bacc.Bacc(
        "TRN2", target_bir_lowering=False, debug=False, num_devices=NCORE,
    )

    def din(name, shape, dtype=F32):
        return nc.dram_tensor(name, list(shape), dtype, kind="ExternalInput").ap()

    gen_w8 = din("gen_w8", (128, 4, V), FP8)      # gen_W * SW, [p, kc, v]
    dec_g8 = din("dec_g8", (128, 4, 128), FP8)    # dec^T * SD, [p, kc, (b,l16)]
    if genb_nz:
        genb = din("genb", (1, V), BF16)
    dec_cl = din("dec_cl", (128, 2048), BF16)     # quad-block-diag dec chunks
    src_cs8 = din("src_cs8", (128, 32, CSW), BF16)  # src^T blocks [p, (q,i,kc), c]
    maskadd = din("maskadd", (128, CSW), BF16)    # 0 / NEG*SCALE additive mask
    gblk = din("gblk", (128, psw // dgat // 16), I16)  # gather block ids
    tw = din("tw", (128, NT * cpt))          # one-hot keys per slot chunk
    iota_in = din("iota_in", (128, TS), F16)      # 0..511 per partition
    ident_in = din("ident_in", (128, 128))        # fp32 identity (transposes)
    if not fast:
        c30 = din("c30", (BL * 30, V), BF16)      # [pv;l;tp] concat per batch
        crel = din("crel", (BL * REL, V), BF16)
    out = nc.dram_tensor("out", [128, V], BF16, kind="ExternalOutput").ap()

    with tile.TileContext(nc) as tc:
        with (
            tc.tile_pool(name="const", bufs=1) as constp,
            tc.tile_pool(name="pgen", bufs=1) as pgenp,
            tc.tile_pool(name="decs", bufs=1) as decp,
            tc.tile_pool(name="genw", bufs=6) as genwp,
            tc.tile_pool(name="cat", bufs=4) as catp,
            tc.tile_pool(name="ohs", bufs=16) as ohsp,
            tc.tile_pool(name="probs", bufs=1) as probsp,
            tc.tile_pool(name="ptcs", bufs=12) as ptcp,
            tc.tile_pool(name="outs", bufs=4) as outp,
        ):
            # preload the Q7 ucode library for ap_gather so the ~130us
            # reload overlaps the gen phase instead of gating the main loop
            nc.gpsimd.load_library(library_config.ap_gather)

            # dummy ap_gather: absorbs the ~30us gpsimd pre-op drain at t=0
            # (GpSimd queue only) so the real gather isn't gated by it
            dgi = constp.tile([16, 1], I16, tag="dgi")
            nc.gpsimd.memset(dgi[:], 0)
            dgd = constp.tile([16, 16], BF16, tag="dgd")
            nc.gpsimd.memset(dgd[:], 0.0)
            dgo = constp.tile([16, 32], BF16, tag="dgo")
            nc.gpsimd.ap_gather(
                dgo[:], dgd[:], dgi[:],
                channels=16, num_elems=8, d=2, num_idxs=16,
            )

            # ---------------- DMA order: copy-block feeders first (they
            # gate probs_cb -> gather -> the whole scatter pipeline)
            deccl_sb = decp.tile([128, 2048], BF16, tag="deccl")
            nc.sync.dma_start(deccl_sb[:], dec_cl[:, :])

            sums = constp.tile([128, NT + 1], F32, tag="sums")
            pgen = pgenp.tile([128, V], BF16)      # unnormalized gen probs / 16

            ln16n = constp.tile([128, 1], F32, tag="ln16n")
            nc.vector.memset(ln16n[:], -LN16)
            ones_sb = decp.tile([1, 128], BF16, tag="ones")
            nc.vector.memset(ones_sb[:], 1.0)

            # ---------------- copy block FIRST: src streams ahead of gen_w8
            # on the same sync ring (one coalesced 2MB DMA per quad wave --
            # 16 small DMAs dribble at ~45GB/s) so probs_cb and the gather
            # overlap the gen phase.
            with (
                tc.tile_pool(name="srcs", bufs=2) as srcp,
                tc.tile_pool(name="cl_ps", bufs=1, space="PSUM") as clps,
            ):
                cps = clps.tile([128, CSW], F32)
                src_w = []
                for q in range(2):
                    sw_ = srcp.tile([128, 16, CSW], BF16, tag="src")
                    nc.sync.dma_start(
                        sw_[:], src_cs8[:, 16 * q:16 * (q + 1), :])
                    src_w.append(sw_)
                maskadd_sb = decp.tile([128, CSW], BF16, tag="maskadd")
                nc.sync.dma_start(maskadd_sb[:], maskadd[:, :])
                dec8_sb = decp.tile([128, 4, 128], FP8, tag="dec8")
                nc.sync.dma_start(dec8_sb[:], dec_g8[:, :, :])
                for q in range(2):
                    for kc in range(16):
                        nc.tensor.matmul(
                            cps[64 * q:64 * (q + 1), :],
                            deccl_sb[:, q * 1024 + kc * 64:
                                     q * 1024 + (kc + 1) * 64],
                            src_w[q][:, kc, :],
                            start=(kc == 0), stop=(kc == 15),
                        )
                # copy-block logits then unnormalized probs/16 + denominator
                lcb = probsp.tile([128, CSW], F32, tag="lcb")
                nc.vector.scalar_tensor_tensor(
                    lcb[:], cps[:], SCALE, maskadd_sb[:],
                    op0=mybir.AluOpType.mult, op1=mybir.AluOpType.add,
                )
                probs_cb = probsp.tile([128, CSW], BF16, tag="pcb")
                nc.scalar.activation(
                    probs_cb[:], lcb[:],
                    mybir.ActivationFunctionType.Exp,
                    bias=ln16n[:, 0:1],
                    accum_out=sums[:, NT:NT + 1],
                )

            # ---------------- remaining small inputs
            gblk_sb = decp.tile([128, psw // dgat // 16], I16, tag="gblk")
            nc.sync.dma_start(gblk_sb[:], gblk[:, :])
            tw_sb = decp.tile([128, NT * cpt], F32, tag="tw")
            nc.sync.dma_start(tw_sb[:], tw[:, :])
            iota512 = constp.tile([128, TS], F16)
            nc.sync.dma_start(iota512[:], iota_in[:, :])
            ident = constp.tile([128, 128], F32)
            nc.sync.dma_start(ident[:], ident_in[:, :])
            identb = constp.tile([128, 128], BF16, tag="identb")
            nc.vector.tensor_copy(identb[:], ident[:])

            # slot-gather the (unnormalized) copy-block probs; native pool op
            gath = probsp.tile([128, psw], BF16, tag="gath")
            nc.gpsimd.ap_gather(
                gath[:], probs_cb[:], gblk_sb[:],
                channels=128, num_elems=CSW // dgat, d=dgat,
                num_idxs=psw // dgat,
            )

            # gen probs: per tile-group DMA, DoubleRow fp8 matmuls,
            # then Exp straight off PSUM with the prefix scale folded in.
            ctx_gen = tc.tile_pool(name="gen_ps", bufs=2, space="PSUM")
            genps = ctx_gen.__enter__()
            for g in range(NT // TG):
                wt = genwp.tile([128, 4, TG * TS], FP8, tag="w")
                nc.sync.dma_start(
                    wt[:], gen_w8[:, :, TG * TS * g:TG * TS * (g + 1)])
                if genb_nz:
                    bt = genwp.tile([1, TG * TS], BF16, tag="bias")
                    nc.sync.dma_start(
                        bt[:], genb[0:1, TG * TS * g:TG * TS * (g + 1)])
                for tt in range(TG):
                    t = TG * g + tt
                    ps = genps.tile([128, TS], F32, tag="gen")
                    for k in range(2):
                        nc.tensor.matmul(
                            ps[:], dec8_sb[:, 2 * k:2 * k + 2, :],
                            wt[:, 2 * k:2 * k + 2, TS * tt:TS * (tt + 1)],
                            start=(k == 0), stop=(k == 1 and not genb_nz),
                            perf_mode=mybir.MatmulPerfMode.DoubleRow,
                        )
                    if genb_nz:
                        # genb pre-scaled by SW*SD on host to match the
                        # fp8 product scale in PSUM
                        nc.tensor.matmul(
                            ps[:], ones_sb[:], bt[:, TS * tt:TS * (tt + 1)],
                            start=False, stop=True,
                        )
                    nc.scalar.activation(
                        pgen[:, TS * t:TS * (t + 1)], ps[:],
                        mybir.ActivationFunctionType.Exp,
                        scale=EXP_GEN,
                        bias=ln16n[:, 0:1],
                        accum_out=sums[:, t:t + 1],
                    )

            ctx_gen.__exit__(None, None, None)

            # ---------------- denominator (of the /16-scaled exps)
            sumexp = constp.tile([128, 1], F32, tag="sumexp")
            nc.vector.tensor_reduce(
                sumexp[:], sums[:], axis=mybir.AxisListType.X,
                op=mybir.AluOpType.add,
            )
            recip = constp.tile([128, 1], F32, tag="recip")
            nc.vector.reciprocal(recip[:], sumexp[:])

            # deeper PSUM rotation for the scatter pipeline: gen/copy-logit
            # banks are idle by now, so it gets 4 cp + 4 transpose banks
            ctx_ps = tc.tile_pool(name="copy_ps", bufs=4, space="PSUM")
            copyps = ctx_ps.__enter__()
            ctx_tr = tc.tile_pool(name="tr_ps", bufs=4, space="PSUM")
            trps = ctx_tr.__enter__()

            if not fast:
                # 1/sum folded into a diagonal for the legacy transpose path
                diagr = constp.tile([128, 128], BF16, tag="diagr")
                nc.vector.tensor_scalar_mul(diagr[:], ident[:], recip[:])
                # quad-block-diag lhsTs for the fixed-60 dense matmuls
                tp0 = trps.tile([128, 128], F32, tag="tr")
                nc.tensor.matmul(tp0[0:64, :], probs_cb[:, 0:64], diagr[:])
                ptfix = probsp.tile([64, 128], BF16, tag="ptfix")
                nc.vector.tensor_copy(ptfix[:], tp0[0:64, :])
                bd30 = probsp.tile([128, 128], BF16, tag="bd30")
                bdrel = probsp.tile([128, 128], BF16, tag="bdrel")
                nc.vector.memset(bd30[:], 0.0)
                nc.vector.memset(bdrel[:], 0.0)
                for q in range(2):
                    for i in range(4):
                        cs = 64 * q + 16 * i
                        nc.sync.dma_start(
                            bd30[30 * i:30 * i + 30, cs:cs + 16],
                            ptfix[0:30, cs:cs + 16])
                        nc.sync.dma_start(
                            bdrel[30 * i:30 * i + 30, cs:cs + 16],
                            ptfix[30:60, cs:cs + 16])

            # ---------------- main output loop
            if fast:
                # fp8 DoubleRow scatter: both 128-row slot chunks of a tile
                # fold into ONE K=256 matmul; gen probs accumulate into the
                # same PSUM via an identity matmul; ScalarE evacuates with
                # the 1/sum folded into its per-partition scale.
                for t in range(NT):
                    cp = copyps.tile([128, TS], F32, tag="cp")
                    ptc2 = ptcp.tile([128, cpt, 128], FP8, tag="ptc")
                    o2 = ohsp.tile([128, cpt, TS], FP8, tag="ohs")
                    for cc in range(cpt):
                        tpp = trps.tile([128, 128], F32, tag="tr")
                        koff = 128 * (t * cpt + cc)
                        nc.tensor.matmul(
                            tpp[:], gath[:, koff:koff + 128], identb[:])
                        if cc % 2 == 0:
                            nc.scalar.copy(ptc2[:, cc, :], tpp[:])
                        else:
                            nc.vector.tensor_copy(ptc2[:, cc, :], tpp[:])
                        nc.vector.tensor_scalar(
                            o2[:, cc, :], iota512[:],
                            tw_sb[:, t * cpt + cc:t * cpt + cc + 1],
                            None, mybir.AluOpType.is_equal,
                        )
                    for pp in range(cpt // 2):
                        nc.tensor.matmul(
                            cp[:], ptc2[:, 2 * pp:2 * pp + 2, :],
                            o2[:, 2 * pp:2 * pp + 2, :],
                            start=(pp == 0), stop=False,
                            perf_mode=mybir.MatmulPerfMode.DoubleRow,
                        )
                    nc.tensor.matmul(
                        cp[:], identb[:], pgen[:, TS * t:TS * (t + 1)],
                        start=False, stop=True,
                    )
                    ot = outp.tile([128, TS], BF16, tag="o")
                    nc.scalar.mul(ot[:], cp[:], recip[:, 0:1])
                    nc.sync.dma_start(out[:, TS * t:TS * (t + 1)], ot[:])

            for t in range(0 if fast else NT):    # legacy (dense) path
                cp = copyps.tile([128, TS], F32, tag="cp")
                for q in range(2):
                    if not fast:
                        cat30 = catp.tile([128, TS], BF16, tag="cat30")
                        catrel = catp.tile([128, TS], BF16, tag="catrel")
                        nc.sync.dma_start(
                            cat30[0:120, :],
                            c30[120 * q:120 * (q + 1), TS * t:TS * (t + 1)])
                        nc.sync.dma_start(
                            catrel[0:120, :],
                            crel[120 * q:120 * (q + 1), TS * t:TS * (t + 1)])
                        nc.tensor.matmul(
                            cp[64 * q:64 * (q + 1), :],
                            bd30[0:120, 64 * q:64 * (q + 1)],
                            cat30[0:120, :],
                            start=True, stop=False,
                        )
                        nc.tensor.matmul(
                            cp[64 * q:64 * (q + 1), :],
                            bdrel[0:120, 64 * q:64 * (q + 1)],
                            catrel[0:120, :],
                            start=False, stop=False,
                        )
                    for s in range(cpq):
                        cc = q * cpq + s            # chunk within this tile
                        tpp = trps.tile([128, 128], F32, tag="tr")
                        koff = 128 * (t * cpt + cc)
                        nc.tensor.matmul(
                            tpp[:],
                            gath[:, koff:koff + 128],
                            diagr[:])
                        ptc = ptcp.tile([128, 128], BF16, tag="ptc")
                        nc.scalar.copy(ptc[:], tpp[:])
                        o = ohsp.tile([128, TS], BF16, tag="ohs")
                        nc.vector.tensor_scalar(
                            o[:], iota512[:],
                            tw_sb[:, t * cpt + cc:t * cpt + cc + 1],
                            None, mybir.AluOpType.is_equal,
                        )
                        nc.tensor.matmul(
                            cp[64 * q:64 * (q + 1), :],
                            ptc[:, 64 * q:64 * (q + 1)],
                            o[:],
                            start=(fast and s == 0), stop=(s == cpq - 1),
                        )
                # out = pgen * recip + copy_psum
                ot = outp.tile([128, TS], BF16, tag="o")
                nc.vector.scalar_tensor_tensor(
                    ot[:], pgen[:, TS * t:TS * (t + 1)], recip[:], cp[:],
                    op0=mybir.AluOpType.mult, op1=mybir.AluOpType.add,
                )
                nc.sync.dma_start(
                    out[:, TS * t:TS * (t + 1)], ot[:])

            ctx_tr.__exit__(None, None, None)
            ctx_ps.__exit__(None, None, None)

    nc.compile()
    _BUILD_CACHE[key] = nc
    return nc


# ================================================================ host prep
def _onehot_idx(mat):
    """Return [B, p] argmax indices if mat rows are exact one-hot, else None."""
    mat = np.asarray(mat)
    idx = mat.argmax(-1)
    if not (np.take_along_axis(mat, idx[..., None], -1) == 1.0).all():
        return None
    if (mat != 0).sum(-1).max() != 1:
        return None
    return idx.astype(np.int64)


def _prep(dec_out, src_hidden, src_mask, pv_m, l, tp, related,
          gen_W, gen_b, context, glo2loc):
    """Numpy-side layout/index prep -> (in_maps, slot, genb_nz, fast)."""
    f32 = np.float32
    dec_out = np.asarray(dec_out, f32)
    src_hidden = np.asarray(src_hidden, f32)
    src_mask = np.asarray(src_mask)
    gen_W = np.asarray(gen_W, f32)
    gen_b = np.asarray(gen_b, f32)
    context = np.asarray(context)
    glo2loc = np.asarray(glo2loc)

    transfer = glo2loc[context].astype(np.int64)          # [B, CTX]
    order = np.argsort(transfer, axis=1, kind="stable")   # [B, CTX]

    oh = [_onehot_idx(m) for m in (pv_m, l, tp, related)]
    fast = all(o is not None for o in oh)

    # scatter entry list per batch, sorted by target v.  Each entry's
    # compact-copy-space column equals its sorted position (+64 in the
    # dense path), so each (batch, v-tile) occupies one consecutive run
    # and the slot gather works on 32-wide blocks.
    ctx_targets = np.take_along_axis(transfer, order, 1)  # sorted ctx targets
    ctx_srcrow = 30 + order                               # original src rows
    if fast:
        fixed_t = np.concatenate(oh, 1)                   # [B, 60]
        fr = np.concatenate([np.arange(30), 286 + np.arange(30)])
        fixed_r = np.tile(fr, (B, 1))
        targets = np.concatenate([fixed_t, ctx_targets], 1)   # [B, 316]
        srcrow = np.concatenate([fixed_r, ctx_srcrow], 1)
        csbase = 0
    else:
        targets, srcrow = ctx_targets, ctx_srcrow
        csbase = 64
    o2 = np.argsort(targets, axis=1, kind="stable")
    targets = np.take_along_axis(targets, o2, 1)
    srcrow = np.take_along_axis(srcrow, o2, 1)
    ne = targets.shape[1]

    tile_of = targets // TS
    within = (targets % TS).astype(np.float32)
    K = np.zeros((B, NT), np.int64)
    np.add.at(K, (np.repeat(np.arange(B), ne), tile_of.ravel()), 1)

    slot = 32 if K.max() <= 32 else 64
    assert K.max() <= slot, f"scatter tile count {K.max()} > {slot}"
    psw = BL * slot * NT
    cpt = BL * slot // 128

    # pad each (batch, tile) run to a multiple of the gather block size
    # so runs start block-aligned in the compact copy space
    dgat = 8
    L = (np.ceil(K / dgat) * dgat).astype(np.int64)       # padded run lengths
    if int(L.sum(1).max()) + 32 > 512 - csbase:
        dgat = 4
        L = (np.ceil(K / dgat) * dgat).astype(np.int64)
    cum8 = np.concatenate(
        [np.zeros((B, 1), np.int64), np.cumsum(L, 1)[:, :-1]], 1) + csbase
    csw = 512
    assert int((cum8[:, -1] + L[:, -1]).max()) + 32 <= csw

    genb_nz = bool(np.any(gen_b != 0.0))

    # per-batch column map: sorted entry i -> its padded-run column
    colpos = np.zeros((B, ne), np.int64)
    for b in range(B):
        rank = np.arange(ne) - np.concatenate(
            [np.zeros(1, np.int64),
             np.cumsum(K[b])])[tile_of[b]]
        colpos[b] = cum8[b][tile_of[b]] + rank

    # src_hidden^T in compact copy-space order + additive mask
    srcT = src_hidden.transpose(0, 2, 1)                  # [B, H, S]
    src_cs = np.zeros((B, H, csw), f32)
    m = src_mask[:, 0, :]                                 # [B, S]
    maskcs = np.zeros((B, csw), np.int64)
    if not fast:
        src_cs[:, :, 0:30] = srcT[:, :, 0:30]
        src_cs[:, :, 30:60] = srcT[:, :, 286:316]
        maskcs[:, 0:30] = m[:, 0:30]
        maskcs[:, 30:60] = m[:, 286:316]
    bi = np.repeat(np.arange(B), ne)
    src_cs[bi, :, colpos.ravel()] = np.take_along_axis(
        srcT, srcrow[:, None, :], 2).transpose(0, 2, 1).reshape(B * ne, H)
    maskcs[bi, colpos.ravel()] = np.take_along_axis(m, srcrow, 1).ravel()

    # one-hot keys per (batch, tile, rank); gather block ids per tile
    tw_bts = np.full((B, NT, slot), -1, np.float32)
    for b in range(B):
        rank = colpos[b] - cum8[b][tile_of[b]]
        tw_bts[b, tile_of[b], rank] = within[b]
    gblk_b = (cum8 // dgat).astype(np.int16)              # [B, NT] block ids
    PADBLK = (csw - 32) // dgat                           # all-masked block

    iota_in = np.tile(np.arange(TS, dtype=np.float16), (128, 1))
    ident_in = np.eye(128, dtype=f32)

    in_maps = []
    # fp8 gen weights: [p, kc, v] = W[kc*128+p, v] * SW
    gen_w8 = np.ascontiguousarray(
        (gen_W.reshape(4, 128, V).transpose(1, 0, 2) * SW).astype(F8))
    genb_s = np.ascontiguousarray(
        (gen_b[None, :] * (SW * SD)).astype(BF)) if genb_nz else None
    if not fast:
        c30_all = np.concatenate(
            [np.asarray(pv_m, f32), np.asarray(l, f32),
             np.asarray(tp, f32)], 1)                     # [B, 30, V]
        crel_all = np.asarray(related, f32)
    for c in range(NCORE):
        gb = slice(c * BL, (c + 1) * BL)
        gbi = np.arange(c * BL, (c + 1) * BL)
        d = dec_out[gb]                                    # [BL, LA, H]

        dec_gx = np.zeros((H, 128), f32)
        for b in range(BL):
            dec_gx[:, 16 * b:16 * b + LA] = d[b].T
        dec_g8 = (dec_gx.reshape(4, 128, 128).transpose(1, 0, 2) * SD)

        dec_cl = np.zeros((128, 2048), f32)
        for q in range(2):
            for kc in range(16):
                lb = 4 * q + kc // 4
                hs = slice(128 * (kc % 4), 128 * (kc % 4 + 1))
                off = q * 1024 + kc * 64 + 16 * (kc // 4)
                dec_cl[:, off:off + LA] = d[lb].T[hs]

        maskadd_c = np.full((128, csw), NEG_S, f32)
        for b in range(BL):
            maskadd_c[16 * b:16 * b + LA, :] = np.where(
                maskcs[c * BL + b] == 1, 0.0, NEG_S)[None, :]

        # gather block ids: slot block k' -> tile k'//(32*8/d) ... each
        # (batch, tile) owns 32/dgat consecutive blocks; a batch's group
        # keeps its own runs, other batches' blocks point at the
        # all-masked pad block (prob == 0 -> block-diag zeros)
        bpt = slot // dgat                  # blocks per (batch, tile)
        nblk = psw // dgat
        gblk_c = np.zeros((128, nblk // 16), np.int16)
        kk2 = np.arange(nblk)
        tile2, sub2b = kk2 // (BL * bpt), kk2 % (BL * bpt)
        bat2, off2 = sub2b // bpt, sub2b % bpt
        for b in range(BL):
            lst = np.where(bat2 == b,
                           gblk_b[c * BL + b, tile2] + off2, PADBLK)
            for p in range(16):
                gblk_c[16 * b + p] = lst[p::16].astype(np.int16)

        # one-hot keys per slot chunk
        tw_c = np.full((128, NT * cpt), -1, np.float32)
        rr = np.arange(128)
        for t in range(NT):
            for cc in range(cpt):
                kk = 128 * (t * cpt + cc) + rr
                sub2 = kk % (BL * slot)
                tw_c[:, t * cpt + cc] = tw_bts[
                    gbi[sub2 // slot], t, sub2 % slot]

        im = dict(
            gen_w8=gen_w8,
            dec_g8=np.ascontiguousarray(dec_g8.astype(F8)),
            dec_cl=np.ascontiguousarray(dec_cl.astype(BF)),
            src_cs8=np.ascontiguousarray(
                src_cs[gb].reshape(2, 4, 4, 128, csw)
                .transpose(3, 0, 1, 2, 4).reshape(128, 32, csw).astype(BF)),
            maskadd=np.ascontiguousarray(maskadd_c.astype(BF)),
            gblk=np.ascontiguousarray(gblk_c),
            tw=np.ascontiguousarray(tw_c),
            iota_in=iota_in,
            ident_in=ident_in,
        )
        if genb_nz:
            im["genb"] = genb_s
        if not fast:
            im.update(
                c30=np.ascontiguousarray(
                    c30_all[gb].reshape(BL * 30, V).astype(BF)),
                crel=np.ascontiguousarray(
                    crel_all[gb].reshape(BL * REL, V).astype(BF)),
            )
        in_maps.append(im)
    return in_maps, slot, genb_nz, fast, csw, dgat


# ================================================================ entry
def kernel(**inputs) -> np.ndarray:
    in_maps, slot, genb_nz, fast, csw, dgat = _prep(**inputs)
    nc = _build(slot, genb_nz, fast, csw, dgat)
    res = run_bass_kernel_spmd(nc, in_maps, core_ids=list(range(NCORE)))
    outs = [np.asarray(res.results[c]["out"]).astype(np.float32)
            .reshape(BL, 16, V)[:, :LA]
            for c in range(NCORE)]
    return np.concatenate(outs, 0)

